# revision 12
# baseline (speedup 1.0000x reference)
"""Trainium2 Bass kernel for nn_BiLinearInteractionLayer.

Math: x:(B=4096, F=32, D=64) f32, W:(P=496, D=64, D=64) f32 (torch Linear
layout: out_e = sum_d in_d * W[e, d]).  For each pair p=(i,j), i<j:
    out[b, p, e] = (sum_d x[b,i,d] * W[p,e,d]) * x[b,j,e]

Strategy (data-parallel over batch, 8 cores x 512 rows):

Native fp32 matmul on the PE costs 4 cycles/column (2 hi/lo passes x 2).
Instead we do our own hi/lo split into fp16 (11-bit mantissa halves ->
~2^-22 combined input precision, fp32 PSUM accumulate) in TWO passes by
exploiting the k=64 contraction:

  pass A (k=128): lhsT = [x_hi; x_lo] stacked on 128 partitions,
                  rhs  = [W_lo; W_hi] -> x_hi@W_lo + x_lo@W_hi in one go
  pass B (k=64, rows 64-127): x_hi @ W_hi, reusing the W_hi rows of the
                  same weight tile and an upper-half replica of x_hi

The dropped x_lo@W_lo term is ~2^-24 relative.  W is pre-scaled by 8 on
the host (power of two, exact) so its fp16 'lo' half stays in normal
range; the elementwise multiply uses x/8 (also exact) to compensate.

Weights are host-pretransposed to WT[d, p*64+e], split into fp16 halves
(offline weight preformatting), and shipped as one (128, P*64) array
with rows [W_lo; W_hi], replicated to every core.  On chip it lives in
one SBUF tile per left-field group so matmuls only wait for their own
slice of the load.

Per 128-row batch tile, per group of 4 left fields: PE-transpose the
fields, split hi/lo (ACT/DVE), shift lo and an x_hi replica to
partitions 64-127 (small GPSIMD SBUF->SBUF DMAs), then immediately run
that group's matmul chunks (<=8 pairs each) and fuse the elementwise
product with the PSUM->SBUF move on DVE against the natively-laid-out
right-field slice of x/8.  One store per left field (contiguous pair
range, ~0.25-0.5MB).  DMA rings are split by stream: stores + x loads on
the Sync HWDGE ring, weight loads on the Scalar ring, SBUF shifts on
GPSIMD SWDGE, so no stream stalls another.
"""
import numpy as np

import concourse.bacc as bacc
import concourse.tile as tile
import concourse.mybir as mybir
from concourse.bass_utils import run_bass_kernel_spmd
from concourse.masks import make_identity

B = 4096
F = 32
D = 64
P = F * (F - 1) // 2  # 496
N_CORES = 8
BL = B // N_CORES     # 512 rows per core
BT = 128              # batch tile (SBUF partitions)
NBT = BL // BT        # 4 batch tiles per core
CHUNK = 8             # pairs per matmul chunk (8*64 = 512 = one PSUM bank)
TGROUP = 4            # left fields per processing group
NLEFT = F - 1         # left fields 0..30

f32 = mybir.dt.float32
f16 = mybir.dt.float16

_nc_cache = None


def _off(i):
    """Pair index of the first pair with left field i."""
    return 31 * i - i * (i - 1) // 2


def _chunks(npair):
    out = []
    c0 = 0
    rem = npair
    while rem > 0:
        if rem > CHUNK:
            take = CHUNK if rem - CHUNK >= 4 or rem - CHUNK == 0 else rem - 4
        else:
            take = rem
        out.append((c0, take))
        c0 += take
        rem -= take
    return out


_GROUPS = [(g0, min(TGROUP, NLEFT - g0)) for g0 in range(0, NLEFT, TGROUP)]


def _build():
    nc = bacc.Bacc("TRN2", target_bir_lowering=False, debug=False,
                   num_devices=N_CORES)
    x_in = nc.dram_tensor("x", [BL, F * D], f32, kind="ExternalInput").ap()
    # rows 0-63: fp16 lo(8*W^T); rows 64-127: fp16 hi(8*W^T)
    wt_in = nc.dram_tensor("wt", [128, P * D], f16, kind="ExternalInput").ap()
    out = nc.dram_tensor("out", [BL, P * D], f32, kind="ExternalOutput").ap()

    with tile.TileContext(nc) as tc:
        with (
            tc.tile_pool(name="consts", bufs=1) as consts,
            tc.tile_pool(name="xp", bufs=2) as xp,
            tc.tile_pool(name="xsp", bufs=2) as xsp,
            tc.tile_pool(name="xtp", bufs=2) as xtp,
            tc.tile_pool(name="xup", bufs=2) as xup,
            tc.tile_pool(name="lop", bufs=2) as lop,
            tc.tile_pool(name="otp", bufs=3) as otp,
            tc.tile_pool(name="pst", bufs=2, space="PSUM") as pst,
            tc.tile_pool(name="psm", bufs=6, space="PSUM") as psm,
        ):
            identity = consts.tile([128, 128], f32)
            make_identity(nc, identity)

            # one weight tile per field group -> matmuls of group g only
            # depend on load g.  All weight loads go on the Scalar HWDGE
            # ring so the Sync ring (x loads + stores) never waits.
            wt_g = []
            for gi, (g0, gn) in enumerate(_GROUPS):
                c0 = _off(g0) * D
                c1 = _off(g0 + gn) * D
                t = consts.tile([128, c1 - c0], f16, tag=f"wt{gi}")
                wt_g.append(t)
                nc.scalar.dma_start(out=t, in_=wt_in[:, c0:c1])

            for bt in range(NBT):
                x_tile = xp.tile([BT, F * D], f32, tag="x")
                nc.sync.dma_start(out=x_tile, in_=x_in[bt * BT:(bt + 1) * BT, :])

                # x/8 for the elementwise side (exact power-of-two scale)
                x_scaled = xsp.tile([BT, F * D], f32, tag="xs")
                nc.scalar.mul(x_scaled, x_tile, 0.125)

                # xT_cross partitions 0-63: fp16 hi of x^T (left fields);
                # partitions 64-127: fp16 lo.  xT_hiU partitions 64-127:
                # replica of hi for pass B (lower half unused).
                xT_cross = xtp.tile([128, NLEFT, BT], f16, tag="xT")
                xT_hiU = xup.tile([128, NLEFT, BT], f16, tag="xU")
                lo_stage = lop.tile([D, NLEFT, BT], f16, tag="lo")

                for gi, (g0, gn) in enumerate(_GROUPS):
                    # pair-transpose: one [128,128] PE transpose covers TWO
                    # adjacent fields -> field g0+2s lands on psum rows
                    # 0-63 ("low"), field g0+2s+1 on rows 64-127 ("up")
                    nlow = (gn + 1) // 2
                    nup = gn // 2
                    evn = slice(g0, g0 + gn, 2)       # low-native fields
                    odd = slice(g0 + 1, g0 + gn, 2)   # up-native fields
                    pt = pst.tile([128, (TGROUP + 1) // 2, BT], f32, tag="tp")
                    for sl in range(nlow):
                        i = g0 + 2 * sl
                        w = 2 * D if sl < nup or 2 * sl + 1 < gn else D
                        nc.tensor.transpose(
                            pt[0:(2 if w == 2 * D else 1) * D, sl],
                            x_tile[:, i * D:i * D + w], identity)
                    # hi = fp16(x^T): low-native direct to partitions 0-63,
                    # up-native direct to partitions 64-127
                    nc.scalar.copy(xT_cross[0:D, evn, :], pt[0:D, :nlow])
                    if nup:
                        nc.scalar.copy(xT_hiU[D:128, odd, :],
                                       pt[D:128, :nup])
                    # lo = fp16(x^T - hi)
                    nc.vector.tensor_sub(
                        lo_stage[:, evn, :], pt[0:D, :nlow],
                        xT_cross[0:D, evn, :])
                    if nup:
                        nc.vector.tensor_sub(
                            xT_cross[D:128, odd, :], pt[D:128, :nup],
                            xT_hiU[D:128, odd, :])
                    # partition shifts (SBUF->SBUF via GPSIMD SWDGE):
                    # low-native: lo up, hi up; up-native: hi down
                    nc.gpsimd.dma_start(out=xT_cross[D:128, evn, :],
                                        in_=lo_stage[:, evn, :])
                    nc.gpsimd.dma_start(out=xT_hiU[D:128, evn, :],
                                        in_=xT_cross[0:D, evn, :])
                    if nup:
                        nc.gpsimd.dma_start(out=xT_cross[0:D, odd, :],
                                            in_=xT_hiU[D:128, odd, :])

                    wt = wt_g[gi]
                    gbase = _off(g0) * D
                    for i in range(g0, g0 + gn):
                        npair = F - 1 - i  # pairs (i, i+1..31), consecutive
                        p0 = _off(i)
                        ot = otp.tile([BT, npair * D], f32, tag="ot")
                        for c0, cn in _chunks(npair):
                            n = cn * D
                            cs = (p0 + c0) * D - gbase
                            pm = psm.tile([BT, n], f32, tag="mm")
                            # pass A: k=128, x_hi@W_lo + x_lo@W_hi
                            nc.tensor.matmul(
                                pm, xT_cross[:, i, :], wt[:, cs:cs + n],
                                start=True, stop=False)
                            # pass B: k=64 on rows 64-127, x_hi@W_hi
                            nc.tensor.matmul(
                                pm, xT_hiU[D:128, i, :], wt[D:128, cs:cs + n],
                                start=False, stop=True)
                            j0 = i + 1 + c0  # right fields j0..j0+cn-1
                            nc.vector.tensor_mul(
                                ot[:, c0 * D:c0 * D + n], pm,
                                x_scaled[:, j0 * D:j0 * D + n])
                        nc.sync.dma_start(
                            out=out[bt * BT:(bt + 1) * BT,
                                    p0 * D:(p0 + npair) * D],
                            in_=ot)
    nc.compile()
    return nc


def _get_nc():
    global _nc_cache
    if _nc_cache is None:
        _nc_cache = _build()
    return _nc_cache


def _prep_weights(W):
    # WT2[d, p*D+e] = 8 * W[p, e, d]; power-of-two scale keeps the fp16
    # lo half in normal range (W ~ N(0,1)/8)
    WT2 = np.ascontiguousarray((W * 8.0).transpose(2, 0, 1)).reshape(D, P * D)
    hi = WT2.astype(np.float16)
    lo = (WT2 - hi.astype(np.float32)).astype(np.float16)
    # rows 0-63 pair with x_hi -> W_lo; rows 64-127 pair with x_lo -> W_hi
    # (and serve as the W_hi operand of pass B)
    return np.ascontiguousarray(np.concatenate([lo, hi], axis=0))


def _run(x, W, trace=False, trace_kwargs=None):
    x = np.ascontiguousarray(np.asarray(x, dtype=np.float32))
    W = np.asarray(W, dtype=np.float32)
    wt = _prep_weights(W)
    xs = x.reshape(N_CORES, BL, F * D)
    in_maps = [{"x": xs[c], "wt": wt} for c in range(N_CORES)]
    res = run_bass_kernel_spmd(_get_nc(), in_maps, list(range(N_CORES)),
                               trace=trace, **(trace_kwargs or {}))
    outs = [res.results[c]["out"].reshape(BL, P, D) for c in range(N_CORES)]
    return np.concatenate(outs, axis=0), res


def kernel(x, W):
    out, _ = _run(x, W)
    return out


# revision 13
# speedup vs baseline: 1.0010x; 1.0010x over previous
"""Trainium2 Bass kernel for nn_BiLinearInteractionLayer.

Math: x:(B=4096, F=32, D=64) f32, W:(P=496, D=64, D=64) f32 (torch Linear
layout: out_e = sum_d in_d * W[e, d]).  For each pair p=(i,j), i<j:
    out[b, p, e] = (sum_d x[b,i,d] * W[p,e,d]) * x[b,j,e]

Strategy (data-parallel over batch, 8 cores x 512 rows):

Native fp32 matmul on the PE costs 4 cycles/column (2 hi/lo passes x 2).
Instead we do our own hi/lo split into fp16 (11-bit mantissa halves ->
~2^-22 combined input precision, fp32 PSUM accumulate) in TWO passes by
exploiting the k=64 contraction:

  pass A (k=128): lhsT = [x_hi; x_lo] stacked on 128 partitions,
                  rhs  = [W_lo; W_hi] -> x_hi@W_lo + x_lo@W_hi in one go
  pass B (k=64, rows 64-127): x_hi @ W_hi, reusing the W_hi rows of the
                  same weight tile and an upper-half replica of x_hi

The dropped x_lo@W_lo term is ~2^-24 relative.  W is pre-scaled by 8 on
the host (power of two, exact) so its fp16 'lo' half stays in normal
range; the elementwise multiply uses x/8 (also exact) to compensate.

Weights are host-pretransposed to WT[d, p*64+e], split into fp16 halves
(offline weight preformatting), and shipped as one (128, P*64) array
with rows [W_lo; W_hi], replicated to every core.  On chip it lives in
one SBUF tile per left-field group so matmuls only wait for their own
slice of the load.

Per 128-row batch tile, per group of 4 left fields: PE-transpose the
fields, split hi/lo (ACT/DVE), shift lo and an x_hi replica to
partitions 64-127 (small GPSIMD SBUF->SBUF DMAs), then immediately run
that group's matmul chunks (<=8 pairs each) and fuse the elementwise
product with the PSUM->SBUF move on DVE against the natively-laid-out
right-field slice of x/8.  One store per left field (contiguous pair
range, ~0.25-0.5MB).  DMA rings are split by stream: stores + x loads on
the Sync HWDGE ring, weight loads on the Scalar ring, SBUF shifts on
GPSIMD SWDGE, so no stream stalls another.
"""
import numpy as np

import concourse.bacc as bacc
import concourse.tile as tile
import concourse.mybir as mybir
from concourse.bass_utils import run_bass_kernel_spmd
from concourse.masks import make_identity

B = 4096
F = 32
D = 64
P = F * (F - 1) // 2  # 496
N_CORES = 8
BL = B // N_CORES     # 512 rows per core
BT = 128              # batch tile (SBUF partitions)
NBT = BL // BT        # 4 batch tiles per core
CHUNK = 8             # pairs per matmul chunk (8*64 = 512 = one PSUM bank)
TGROUP = 4            # left fields per processing group
NLEFT = F - 1         # left fields 0..30

f32 = mybir.dt.float32
f16 = mybir.dt.bfloat16

_nc_cache = None


def _off(i):
    """Pair index of the first pair with left field i."""
    return 31 * i - i * (i - 1) // 2


def _chunks(npair):
    out = []
    c0 = 0
    rem = npair
    while rem > 0:
        if rem > CHUNK:
            take = CHUNK if rem - CHUNK >= 4 or rem - CHUNK == 0 else rem - 4
        else:
            take = rem
        out.append((c0, take))
        c0 += take
        rem -= take
    return out


_GROUPS = [(g0, min(TGROUP, NLEFT - g0)) for g0 in range(0, NLEFT, TGROUP)]


def _build():
    nc = bacc.Bacc("TRN2", target_bir_lowering=False, debug=False,
                   num_devices=N_CORES)
    x_in = nc.dram_tensor("x", [BL, F * D], f32, kind="ExternalInput").ap()
    # rows 0-63: fp16 lo(8*W^T); rows 64-127: fp16 hi(8*W^T)
    wt_in = nc.dram_tensor("wt", [128, P * D], f16, kind="ExternalInput").ap()
    out = nc.dram_tensor("out", [BL, P * D], f32, kind="ExternalOutput").ap()

    with tile.TileContext(nc) as tc:
        with (
            tc.tile_pool(name="consts", bufs=1) as consts,
            tc.tile_pool(name="xp", bufs=2) as xp,
            tc.tile_pool(name="xsp", bufs=2) as xsp,
            tc.tile_pool(name="xtp", bufs=2) as xtp,
            tc.tile_pool(name="xup", bufs=2) as xup,
            tc.tile_pool(name="lop", bufs=2) as lop,
            tc.tile_pool(name="otp", bufs=3) as otp,
            tc.tile_pool(name="pst", bufs=2, space="PSUM") as pst,
            tc.tile_pool(name="psm", bufs=6, space="PSUM") as psm,
        ):
            identity = consts.tile([128, 128], f32)
            make_identity(nc, identity)

            # one weight tile per field group -> matmuls of group g only
            # depend on load g.  All weight loads go on the Scalar HWDGE
            # ring so the Sync ring (x loads + stores) never waits.
            wt_g = []
            for gi, (g0, gn) in enumerate(_GROUPS):
                c0 = _off(g0) * D
                c1 = _off(g0 + gn) * D
                t = consts.tile([128, c1 - c0], f16, tag=f"wt{gi}")
                wt_g.append(t)
                nc.scalar.dma_start(out=t, in_=wt_in[:, c0:c1])

            for bt in range(NBT):
                x_tile = xp.tile([BT, F * D], f32, tag="x")
                nc.sync.dma_start(out=x_tile, in_=x_in[bt * BT:(bt + 1) * BT, :])

                # x/8 for the elementwise side (exact power-of-two scale)
                x_scaled = xsp.tile([BT, F * D], f32, tag="xs")
                nc.scalar.mul(x_scaled, x_tile, 0.125)

                # xT_cross partitions 0-63: fp16 hi of x^T (left fields);
                # partitions 64-127: fp16 lo.  xT_hiU partitions 64-127:
                # replica of hi for pass B (lower half unused).
                xT_cross = xtp.tile([128, NLEFT, BT], f16, tag="xT")
                xT_hiU = xup.tile([128, NLEFT, BT], f16, tag="xU")
                lo_stage = lop.tile([D, NLEFT, BT], f16, tag="lo")

                for gi, (g0, gn) in enumerate(_GROUPS):
                    # pair-transpose: one [128,128] PE transpose covers TWO
                    # adjacent fields -> field g0+2s lands on psum rows
                    # 0-63 ("low"), field g0+2s+1 on rows 64-127 ("up")
                    nlow = (gn + 1) // 2
                    nup = gn // 2
                    evn = slice(g0, g0 + gn, 2)       # low-native fields
                    odd = slice(g0 + 1, g0 + gn, 2)   # up-native fields
                    pt = pst.tile([128, (TGROUP + 1) // 2, BT], f32, tag="tp")
                    for sl in range(nlow):
                        i = g0 + 2 * sl
                        w = 2 * D if sl < nup or 2 * sl + 1 < gn else D
                        nc.tensor.transpose(
                            pt[0:(2 if w == 2 * D else 1) * D, sl],
                            x_tile[:, i * D:i * D + w], identity)
                    # hi = fp16(x^T): low-native direct to partitions 0-63,
                    # up-native direct to partitions 64-127
                    nc.scalar.copy(xT_cross[0:D, evn, :], pt[0:D, :nlow])
                    if nup:
                        nc.scalar.copy(xT_hiU[D:128, odd, :],
                                       pt[D:128, :nup])
                    # lo = fp16(x^T - hi)
                    nc.vector.tensor_sub(
                        lo_stage[:, evn, :], pt[0:D, :nlow],
                        xT_cross[0:D, evn, :])
                    if nup:
                        nc.vector.tensor_sub(
                            xT_cross[D:128, odd, :], pt[D:128, :nup],
                            xT_hiU[D:128, odd, :])
                    # partition shifts (SBUF->SBUF via GPSIMD SWDGE):
                    # low-native: lo up, hi up; up-native: hi down
                    nc.gpsimd.dma_start(out=xT_cross[D:128, evn, :],
                                        in_=lo_stage[:, evn, :])
                    nc.gpsimd.dma_start(out=xT_hiU[D:128, evn, :],
                                        in_=xT_cross[0:D, evn, :])
                    if nup:
                        nc.gpsimd.dma_start(out=xT_cross[0:D, odd, :],
                                            in_=xT_hiU[D:128, odd, :])

                    wt = wt_g[gi]
                    gbase = _off(g0) * D
                    for i in range(g0, g0 + gn):
                        npair = F - 1 - i  # pairs (i, i+1..31), consecutive
                        p0 = _off(i)
                        ot = otp.tile([BT, npair * D], f32, tag="ot")
                        for c0, cn in _chunks(npair):
                            n = cn * D
                            cs = (p0 + c0) * D - gbase
                            pm = psm.tile([BT, n], f32, tag="mm")
                            # pass A: k=128, x_hi@W_lo + x_lo@W_hi
                            nc.tensor.matmul(
                                pm, xT_cross[:, i, :], wt[:, cs:cs + n],
                                start=True, stop=False)
                            # pass B: k=64 on rows 64-127, x_hi@W_hi
                            nc.tensor.matmul(
                                pm, xT_hiU[D:128, i, :], wt[D:128, cs:cs + n],
                                start=False, stop=True)
                            j0 = i + 1 + c0  # right fields j0..j0+cn-1
                            nc.vector.tensor_mul(
                                ot[:, c0 * D:c0 * D + n], pm,
                                x_scaled[:, j0 * D:j0 * D + n])
                        nc.sync.dma_start(
                            out=out[bt * BT:(bt + 1) * BT,
                                    p0 * D:(p0 + npair) * D],
                            in_=ot)
    nc.compile()
    return nc


def _get_nc():
    global _nc_cache
    if _nc_cache is None:
        _nc_cache = _build()
    return _nc_cache


def _prep_weights(W):
    # WT2[d, p*D+e] = 8 * W[p, e, d]; power-of-two scale keeps the fp16
    # lo half in normal range (W ~ N(0,1)/8)
    WT2 = np.ascontiguousarray((W * 8.0).transpose(2, 0, 1)).reshape(D, P * D)
    import ml_dtypes
    hi = WT2.astype(ml_dtypes.bfloat16)
    lo = (WT2 - hi.astype(np.float32)).astype(ml_dtypes.bfloat16)
    # rows 0-63 pair with x_hi -> W_lo; rows 64-127 pair with x_lo -> W_hi
    # (and serve as the W_hi operand of pass B)
    return np.ascontiguousarray(np.concatenate([lo, hi], axis=0))


def _run(x, W, trace=False, trace_kwargs=None):
    x = np.ascontiguousarray(np.asarray(x, dtype=np.float32))
    W = np.asarray(W, dtype=np.float32)
    wt = _prep_weights(W)
    xs = x.reshape(N_CORES, BL, F * D)
    in_maps = [{"x": xs[c], "wt": wt} for c in range(N_CORES)]
    res = run_bass_kernel_spmd(_get_nc(), in_maps, list(range(N_CORES)),
                               trace=trace, **(trace_kwargs or {}))
    outs = [res.results[c]["out"].reshape(BL, P, D) for c in range(N_CORES)]
    return np.concatenate(outs, axis=0), res


def kernel(x, W):
    out, _ = _run(x, W)
    return out


# revision 15
# speedup vs baseline: 1.0114x; 1.0105x over previous
"""Trainium2 Bass kernel for nn_BiLinearInteractionLayer.

Math: x:(B=4096, F=32, D=64) f32, W:(P=496, D=64, D=64) f32 (torch Linear
layout: out_e = sum_d in_d * W[e, d]).  For each pair p=(i,j), i<j:
    out[b, p, e] = (sum_d x[b,i,d] * W[p,e,d]) * x[b,j,e]

Strategy (data-parallel over batch, 8 cores x 512 rows):

Native fp32 matmul on the PE costs 4 cycles/column (2 hi/lo passes x 2).
Instead we do our own hi/lo split into fp16 (11-bit mantissa halves ->
~2^-22 combined input precision, fp32 PSUM accumulate) in TWO passes by
exploiting the k=64 contraction:

  pass A (k=128): lhsT = [x_hi; x_lo] stacked on 128 partitions,
                  rhs  = [W_lo; W_hi] -> x_hi@W_lo + x_lo@W_hi in one go
  pass B (k=64, rows 64-127): x_hi @ W_hi, reusing the W_hi rows of the
                  same weight tile and an upper-half replica of x_hi

The dropped x_lo@W_lo term is ~2^-24 relative.  W is pre-scaled by 8 on
the host (power of two, exact) so its fp16 'lo' half stays in normal
range; the elementwise multiply uses x/8 (also exact) to compensate.

Weights are host-pretransposed to WT[d, p*64+e], split into fp16 halves
(offline weight preformatting), and shipped as one (128, P*64) array
with rows [W_lo; W_hi], replicated to every core.  On chip it lives in
one SBUF tile per left-field group so matmuls only wait for their own
slice of the load.

Per 128-row batch tile, per group of 4 left fields: PE-transpose the
fields, split hi/lo (ACT/DVE), shift lo and an x_hi replica to
partitions 64-127 (small GPSIMD SBUF->SBUF DMAs), then immediately run
that group's matmul chunks (<=8 pairs each) and fuse the elementwise
product with the PSUM->SBUF move on DVE against the natively-laid-out
right-field slice of x/8.  One store per left field (contiguous pair
range, ~0.25-0.5MB).  DMA rings are split by stream: stores + x loads on
the Sync HWDGE ring, weight loads on the Scalar ring, SBUF shifts on
GPSIMD SWDGE, so no stream stalls another.
"""
import numpy as np

import concourse.bacc as bacc
import concourse.tile as tile
import concourse.mybir as mybir
from concourse.bass_utils import run_bass_kernel_spmd
from concourse.masks import make_identity

B = 4096
F = 32
D = 64
P = F * (F - 1) // 2  # 496
N_CORES = 8
BL = B // N_CORES     # 512 rows per core
BT = 128              # batch tile (SBUF partitions)
NBT = BL // BT        # 4 batch tiles per core
CHUNK = 8             # pairs per matmul chunk (8*64 = 512 = one PSUM bank)
TGROUP = 4            # left fields per processing group
NLEFT = F - 1         # left fields 0..30

f32 = mybir.dt.float32
f16 = mybir.dt.bfloat16

_nc_cache = None


def _off(i):
    """Pair index of the first pair with left field i."""
    return 31 * i - i * (i - 1) // 2


def _chunks(npair):
    out = []
    c0 = 0
    rem = npair
    while rem > 0:
        if rem > CHUNK:
            take = CHUNK if rem - CHUNK >= 4 or rem - CHUNK == 0 else rem - 4
        else:
            take = rem
        out.append((c0, take))
        c0 += take
        rem -= take
    return out


_GROUPS = [(g0, min(TGROUP, NLEFT - g0)) for g0 in range(0, NLEFT, TGROUP)]


def _build():
    nc = bacc.Bacc("TRN2", target_bir_lowering=False, debug=False,
                   num_devices=N_CORES)
    x_in = nc.dram_tensor("x", [BL, F * D], f32, kind="ExternalInput").ap()
    # rows 0-63: fp16 lo(8*W^T); rows 64-127: fp16 hi(8*W^T)
    wt_in = nc.dram_tensor("wt", [128, P * D], f16, kind="ExternalInput").ap()
    out = nc.dram_tensor("out", [BL, P * D], f32, kind="ExternalOutput").ap()

    with tile.TileContext(nc) as tc:
        with (
            tc.tile_pool(name="consts", bufs=1) as consts,
            tc.tile_pool(name="xp", bufs=2) as xp,
            tc.tile_pool(name="xsp", bufs=2) as xsp,
            tc.tile_pool(name="xtp", bufs=2) as xtp,
            tc.tile_pool(name="xup", bufs=2) as xup,
            tc.tile_pool(name="lop", bufs=2) as lop,
            tc.tile_pool(name="otp", bufs=3) as otp,
            tc.tile_pool(name="pst", bufs=2, space="PSUM") as pst,
            tc.tile_pool(name="psm", bufs=6, space="PSUM") as psm,
        ):
            identity = consts.tile([128, 128], f32)
            make_identity(nc, identity)

            # one weight tile per field group -> matmuls of group g only
            # depend on load g.  Weight loads go on the Sync HWDGE ring
            # AFTER bt0's x load (issuing them from ACT would block the
            # scalar engine's compute stream behind 8MB of DMA issue).
            wt_g = []
            for gi, (g0, gn) in enumerate(_GROUPS):
                c0 = _off(g0) * D
                c1 = _off(g0 + gn) * D
                t = consts.tile([128, c1 - c0], f16, tag=f"wt{gi}")
                wt_g.append(t)

            for bt in range(NBT):
                x_tile = xp.tile([BT, F * D], f32, tag="x")
                nc.sync.dma_start(out=x_tile, in_=x_in[bt * BT:(bt + 1) * BT, :])
                if bt == 0:
                    for gi, (g0, gn) in enumerate(_GROUPS):
                        c0 = _off(g0) * D
                        c1 = _off(g0 + gn) * D
                        nc.sync.dma_start(out=wt_g[gi], in_=wt_in[:, c0:c1])

                # x/8 for the elementwise side (exact power-of-two scale)
                x_scaled = xsp.tile([BT, F * D], f32, tag="xs")
                nc.scalar.mul(x_scaled, x_tile, 0.125)

                # xT_cross partitions 0-63: fp16 hi of x^T (left fields);
                # partitions 64-127: fp16 lo.  xT_hiU partitions 64-127:
                # replica of hi for pass B (lower half unused).
                xT_cross = xtp.tile([128, NLEFT, BT], f16, tag="xT")
                xT_hiU = xup.tile([128, NLEFT, BT], f16, tag="xU")
                lo_stage = lop.tile([D, NLEFT, BT], f16, tag="lo")

                for gi, (g0, gn) in enumerate(_GROUPS):
                    # pair-transpose: one [128,128] PE transpose covers TWO
                    # adjacent fields -> field g0+2s lands on psum rows
                    # 0-63 ("low"), field g0+2s+1 on rows 64-127 ("up")
                    nlow = (gn + 1) // 2
                    nup = gn // 2
                    evn = slice(g0, g0 + gn, 2)       # low-native fields
                    odd = slice(g0 + 1, g0 + gn, 2)   # up-native fields
                    pt = pst.tile([128, (TGROUP + 1) // 2, BT], f32, tag="tp")
                    for sl in range(nlow):
                        i = g0 + 2 * sl
                        w = 2 * D if sl < nup or 2 * sl + 1 < gn else D
                        nc.tensor.transpose(
                            pt[0:(2 if w == 2 * D else 1) * D, sl],
                            x_tile[:, i * D:i * D + w], identity)
                    # hi = fp16(x^T): low-native direct to partitions 0-63,
                    # up-native direct to partitions 64-127
                    nc.scalar.copy(xT_cross[0:D, evn, :], pt[0:D, :nlow])
                    if nup:
                        nc.scalar.copy(xT_hiU[D:128, odd, :],
                                       pt[D:128, :nup])
                    # lo = fp16(x^T - hi)
                    nc.vector.tensor_sub(
                        lo_stage[:, evn, :], pt[0:D, :nlow],
                        xT_cross[0:D, evn, :])
                    if nup:
                        nc.vector.tensor_sub(
                            xT_cross[D:128, odd, :], pt[D:128, :nup],
                            xT_hiU[D:128, odd, :])
                    # partition shifts (SBUF->SBUF via GPSIMD SWDGE):
                    # low-native: lo up, hi up; up-native: hi down
                    nc.gpsimd.dma_start(out=xT_cross[D:128, evn, :],
                                        in_=lo_stage[:, evn, :])
                    nc.gpsimd.dma_start(out=xT_hiU[D:128, evn, :],
                                        in_=xT_cross[0:D, evn, :])
                    if nup:
                        nc.gpsimd.dma_start(out=xT_cross[0:D, odd, :],
                                            in_=xT_hiU[D:128, odd, :])

                    wt = wt_g[gi]
                    gbase = _off(g0) * D
                    for i in range(g0, g0 + gn):
                        npair = F - 1 - i  # pairs (i, i+1..31), consecutive
                        p0 = _off(i)
                        ot = otp.tile([BT, npair * D], f32, tag="ot")
                        for c0, cn in _chunks(npair):
                            n = cn * D
                            cs = (p0 + c0) * D - gbase
                            pm = psm.tile([BT, n], f32, tag="mm")
                            # pass A: k=128, x_hi@W_lo + x_lo@W_hi
                            nc.tensor.matmul(
                                pm, xT_cross[:, i, :], wt[:, cs:cs + n],
                                start=True, stop=False)
                            # pass B: k=64 on rows 64-127, x_hi@W_hi
                            nc.tensor.matmul(
                                pm, xT_hiU[D:128, i, :], wt[D:128, cs:cs + n],
                                start=False, stop=True)
                            j0 = i + 1 + c0  # right fields j0..j0+cn-1
                            nc.vector.tensor_mul(
                                ot[:, c0 * D:c0 * D + n], pm,
                                x_scaled[:, j0 * D:j0 * D + n])
                        nc.sync.dma_start(
                            out=out[bt * BT:(bt + 1) * BT,
                                    p0 * D:(p0 + npair) * D],
                            in_=ot)
    nc.compile()
    return nc


def _get_nc():
    global _nc_cache
    if _nc_cache is None:
        _nc_cache = _build()
    return _nc_cache


def _prep_weights(W):
    # WT2[d, p*D+e] = 8 * W[p, e, d]; power-of-two scale keeps the fp16
    # lo half in normal range (W ~ N(0,1)/8)
    WT2 = np.ascontiguousarray((W * 8.0).transpose(2, 0, 1)).reshape(D, P * D)
    import ml_dtypes
    hi = WT2.astype(ml_dtypes.bfloat16)
    lo = (WT2 - hi.astype(np.float32)).astype(ml_dtypes.bfloat16)
    # rows 0-63 pair with x_hi -> W_lo; rows 64-127 pair with x_lo -> W_hi
    # (and serve as the W_hi operand of pass B)
    return np.ascontiguousarray(np.concatenate([lo, hi], axis=0))


def _run(x, W, trace=False, trace_kwargs=None):
    x = np.ascontiguousarray(np.asarray(x, dtype=np.float32))
    W = np.asarray(W, dtype=np.float32)
    wt = _prep_weights(W)
    xs = x.reshape(N_CORES, BL, F * D)
    in_maps = [{"x": xs[c], "wt": wt} for c in range(N_CORES)]
    res = run_bass_kernel_spmd(_get_nc(), in_maps, list(range(N_CORES)),
                               trace=trace, **(trace_kwargs or {}))
    outs = [res.results[c]["out"].reshape(BL, P, D) for c in range(N_CORES)]
    return np.concatenate(outs, axis=0), res


def kernel(x, W):
    out, _ = _run(x, W)
    return out
